# revision 6
# baseline (speedup 1.0000x reference)
"""Trainium2 Bass kernel for nn_Loss_62929860821403.

Computes, for features [262144, 256] f32, labels [262144] i64, center [10, 256] f32:
  intra = mean_i ||f_i - center[labels_i]||                (clipped, scalar)
  inter = 1 / clip(2 * ||c2[8] - c2[9]||)                  (scalar)
         where c2 = (center + segment_sum(features)) / max(counts, 1)
  plus center passthrough  -> returns (intra, inter, center)

Strategy (8 cores, data-parallel over batch, 32768 rows/core):
  - Stream features f32 via HWDGE in 1MB chunks ([128, 8, 256], rows on partitions).
  - Per 128-row tile, TensorE gathers the labeled center row with a tiny
    onehot^T x center matmul (bf16 weights, K=10), and accumulates class-8/9
    segment sums with a onehot89^T x rows matmul (M=2).
  - d = f - center[label] is produced either on TensorE (identity-matmul +
    negated-center accumulate, "I" tiles) or on VectorE (subtract, "V"/"A" tiles).
  - Per-row sum of squares via fused ScalarE activation(Square, accum_out) or
    VectorE tensor_tensor_reduce, split across engines to stay under the DMA roofline.
  - Host: sqrt + mean (intra), counts/bincount + last-pair formula (inter).
"""

import numpy as np
import ml_dtypes

B, D, C = 262144, 256, 10
NCORES = 8
R = B // NCORES            # rows per core = 32768
P = 128                    # partitions
T = R // P                 # 256 tiles per core
CHUNK = 8                  # row-tiles per feature DMA (1 MB)
NCHUNK = T // CHUNK        # 32
OHT_SPLIT = 8              # split onehotT load into this many DMAs
EPS_MIN, EPS_MAX = 1e-16, 1e16

# Per-chunk tile pattern:
#  'I': TensorE builds d in PSUM (identity mm + (-center) mm); ScalarE Square+accum
#  'V': VectorE subtract -> d16; VectorE tensor_tensor_reduce for sum of squares
#  'A': VectorE subtract -> d16; ScalarE Square+accum
PATTERN = ['I', 'I', 'V', 'I', 'A', 'I', 'V', 'I']
assert len(PATTERN) == CHUNK

BF16 = ml_dtypes.bfloat16

_cache = {}


def _build_module():
    import concourse.bacc as bacc
    import concourse.tile as tile
    import concourse.bass as bass
    from concourse import mybir

    f32 = mybir.dt.float32
    bf16 = mybir.dt.bfloat16

    nc = bacc.Bacc(
        "TRN2",
        target_bir_lowering=False,
        debug=False,
        enable_asserts=False,
        num_devices=NCORES,
    )

    feat = nc.dram_tensor("feat", [NCHUNK, CHUNK, P, D], f32, kind="ExternalInput")
    ohT = nc.dram_tensor("ohT", [C, T, P], bf16, kind="ExternalInput")
    oh89 = nc.dram_tensor("oh89", [P, T, 2], bf16, kind="ExternalInput")
    ident = nc.dram_tensor("ident", [P, P], bf16, kind="ExternalInput")
    negc = nc.dram_tensor("negc", [C, D], bf16, kind="ExternalInput")
    posc = nc.dram_tensor("posc", [C, D], bf16, kind="ExternalInput")
    own2a = nc.dram_tensor("own2a", [P, T], f32, kind="ExternalOutput")
    own2d = nc.dram_tensor("own2d", [P, T], f32, kind="ExternalOutput")
    s89 = nc.dram_tensor("s89", [2, D], f32, kind="ExternalOutput")

    Sq = mybir.ActivationFunctionType.Square
    Alu = mybir.AluOpType

    with tile.TileContext(nc) as tc:
        with (
            tc.tile_pool(name="fpool", bufs=3) as fpool,
            tc.tile_pool(name="w16", bufs=3) as w16,
            tc.tile_pool(name="scrp", bufs=3) as scrp,
            tc.tile_pool(name="singles", bufs=1) as singles,
            tc.tile_pool(name="ppd", bufs=3, space="PSUM") as ppd,
            tc.tile_pool(name="ppg", bufs=2, space="PSUM") as ppg,
            tc.tile_pool(name="pps", bufs=1, space="PSUM") as pps,
        ):
            # ---- constants / per-core label data ----
            TS = T // OHT_SPLIT
            ohT_sb = []
            for i in range(OHT_SPLIT):
                tsb = singles.tile([C, TS, P], bf16, tag=f"ohT{i}")
                nc.sync.dma_start(out=tsb[:], in_=ohT[:, i * TS:(i + 1) * TS, :])
                ohT_sb.append(tsb)
            oh89_sb = singles.tile([P, T, 2], bf16, tag="oh89")
            nc.sync.dma_start(out=oh89_sb[:], in_=oh89[:])
            I_sb = singles.tile([P, P], bf16, tag="ident")
            nc.sync.dma_start(out=I_sb[:], in_=ident[:])
            negc_sb = singles.tile([C, D], bf16, tag="negc")
            nc.sync.dma_start(out=negc_sb[:], in_=negc[:])
            posc_sb = singles.tile([C, D], bf16, tag="posc")
            nc.sync.dma_start(out=posc_sb[:], in_=posc[:])

            own2a_sb = singles.tile([P, T], f32, tag="own2a")
            own2d_sb = singles.tile([P, T], f32, tag="own2d")
            nc.vector.memset(own2a_sb[:], 0.0)
            nc.vector.memset(own2d_sb[:], 0.0)

            ps = pps.tile([2, D], f32, tag="s89acc")

            for ch in range(NCHUNK):
                ftile = fpool.tile([P, CHUNK, D], f32, tag="f")
                nc.sync.dma_start(
                    out=ftile[:], in_=feat[ch].rearrange("t p d -> p t d")
                )
                for k in range(CHUNK):
                    t = ch * CHUNK + k
                    first, last = (t == 0), (t == T - 1)
                    fsl = ftile[:, k, :]
                    ohT_t = ohT_sb[t // TS][:, t % TS, :]
                    oh89_t = oh89_sb[:, t, :]
                    kind = PATTERN[k]
                    if kind == 'I':
                        f16 = w16.tile([P, D], bf16, tag="f16")
                        nc.gpsimd.tensor_copy(f16[:], fsl)
                        pd = ppd.tile([P, D], f32, tag="d")
                        nc.tensor.matmul(pd[:], I_sb[:], f16[:], start=True, stop=False)
                        nc.tensor.matmul(pd[:], ohT_t, negc_sb[:], start=False, stop=True)
                        scr = scrp.tile([P, D], bf16, tag="scr")
                        nc.scalar.activation(
                            scr[:], pd[:], Sq, accum_out=own2a_sb[:, t:t + 1]
                        )
                        nc.tensor.matmul(
                            ps[:], oh89_t, f16[:],
                            start=first, stop=last, skip_group_check=True,
                        )
                    else:
                        pg = ppg.tile([P, D], f32, tag="g")
                        nc.tensor.matmul(pg[:], ohT_t, posc_sb[:], start=True, stop=True)
                        d16 = w16.tile([P, D], bf16, tag="d16")
                        nc.vector.tensor_tensor(d16[:], fsl, pg[:], Alu.subtract)
                        scr = scrp.tile([P, D], bf16, tag="scr")
                        if kind == 'V':
                            nc.vector.tensor_mul(scr[:], d16[:], d16[:])
                            nc.vector.tensor_reduce(
                                own2d_sb[:, t:t + 1], scr[:],
                                mybir.AxisListType.X, Alu.add,
                            )
                        else:
                            nc.scalar.activation(
                                scr[:], d16[:], Sq, accum_out=own2a_sb[:, t:t + 1]
                            )
                        nc.tensor.matmul(
                            ps[:], oh89_t, d16[:],
                            start=first, stop=last, skip_group_check=True,
                        )

            s89_sb = singles.tile([2, D], f32, tag="s89sb")
            nc.vector.tensor_copy(s89_sb[:], ps[:])
            nc.sync.dma_start(out=own2a[:], in_=own2a_sb[:])
            nc.sync.dma_start(out=own2d[:], in_=own2d_sb[:])
            nc.sync.dma_start(out=s89[:], in_=s89_sb[:])

    nc.compile()
    return nc


def _prep_inputs(features, labels, center):
    """Build per-core in_maps. labels: int64 [B]; features f32 [B, D]."""
    labels = np.asarray(labels).astype(np.int32)
    features = np.ascontiguousarray(np.asarray(features, dtype=np.float32))
    center = np.asarray(center, dtype=np.float32)

    ident = np.eye(P, dtype=BF16)
    negc = (-center).astype(BF16)
    posc = center.astype(BF16)

    in_maps = []
    for c in range(NCORES):
        lab = labels[c * R:(c + 1) * R]
        f = features[c * R:(c + 1) * R].reshape(NCHUNK, CHUNK, P, D)
        L = lab.reshape(T, P)                       # L[t, p] = label of row t*P+p
        ohT_np = (L[None, :, :] == np.arange(C)[:, None, None]).astype(BF16)  # [C,T,P]
        oh89_np = (L.T[:, :, None] == np.array([8, 9])[None, None, :]).astype(BF16)
        in_maps.append({
            "feat": f,
            "ohT": ohT_np,
            "oh89": np.ascontiguousarray(oh89_np),
            "ident": ident,
            "negc": negc,
            "posc": posc,
        })
    return in_maps, labels, center


LAST_RUN_INFO = {}


def kernel(features, labels, center):
    from concourse.bass_utils import run_bass_kernel_spmd
    import os

    in_maps, labels_i32, center_f32 = _prep_inputs(features, labels, center)

    if "nc" not in _cache:
        _cache["nc"] = _build_module()
    nc = _cache["nc"]

    trace = bool(int(os.environ.get("KERNEL_TRACE", "0")))
    res = run_bass_kernel_spmd(
        nc, in_maps, core_ids=list(range(NCORES)), trace=trace,
    )
    LAST_RUN_INFO["exec_time_ns"] = res.exec_time_ns
    LAST_RUN_INFO["results"] = res

    # ---- host-side assembly ----
    # which columns (tiles) were written by ACT vs DVE
    act_k = np.array([PATTERN[k] in ('I', 'A') for k in range(CHUNK)])
    act_col = np.tile(act_k, NCHUNK)                # [T] bool
    f32path_k = np.array([PATTERN[k] != 'I' for k in range(CHUNK)])
    f32path_col = np.tile(f32path_k, NCHUNK)        # tiles whose sums used d16

    posc_f32 = center_f32[[8, 9]].astype(BF16).astype(np.float64)  # bf16 center actually used

    own2_sum_sqrt = 0.0
    sums89 = np.zeros((2, D), dtype=np.float64)
    for c in range(NCORES):
        r = res.results[c]
        own2 = np.where(act_col[None, :], r["own2a"], r["own2d"]).astype(np.float64)
        own2 = np.maximum(own2, 0.0)
        own = np.sqrt(own2)
        own = np.clip(own, EPS_MIN, EPS_MAX)
        own2_sum_sqrt += own.sum()

        lab = labels_i32[c * R:(c + 1) * R].reshape(T, P)
        f32rows = lab[f32path_col]                   # labels of f32-path rows
        n8 = np.float64((f32rows == 8).sum())
        n9 = np.float64((f32rows == 9).sum())
        sums89 += r["s89"].astype(np.float64)
        sums89[0] += n8 * posc_f32[0]
        sums89[1] += n9 * posc_f32[1]

    intra = np.float32(own2_sum_sqrt / B)

    counts = np.bincount(labels_i32, minlength=C).astype(np.float64)
    c2_8 = (center_f32[8].astype(np.float64) + sums89[0]) / max(counts[8], 1.0)
    c2_9 = (center_f32[9].astype(np.float64) + sums89[1]) / max(counts[9], 1.0)
    last = np.sqrt(max(((c2_8 - c2_9) ** 2).sum(), 0.0))
    inter = np.float32(1.0 / np.clip(2.0 * last, EPS_MIN, EPS_MAX))

    return (intra, inter, center_f32)
